# revision 22
# baseline (speedup 1.0000x reference)
# kernel.py — AgentAttention on 8 Trainium2 NeuronCores (self-contained).
#
# Problem (per batch b, head h):
#   qq  = softmax(q @ a, axis=-1)            # [N, d] over agents d
#   kk  = softmax(a @ k, axis=-1)            # [d, N] over keys N
#   out = qq @ (kk @ v)                      # [N, d]
# Shapes: q [8,16,2048,128], a [8,16,128,128], k [8,16,128,2048],
#         v [8,16,2048,128]; d == n_agents == 128.
#
# Sharding: batch dimension (8) across the 8 cores; each core computes its
# 16 heads independently (pure data parallel, no collectives).
#
# Per-head device algorithm (all matmuls contract over the partition dim):
#   aT   = transpose(a)                        (PE transpose, fp32)
#   s2T  = (a @ k)^T  [m, j] via lhsT=k-chunk, rhs=aT        (fp32)
#   e2   = exp(s2T)  -> bf16   (no max subtraction: |logit| <~ 70 < 88.7,
#                               exp fits fp32/bf16 range exactly)
#   agg|S = sum_m e2[m,:]^T @ [v_m | 1]  (bf16 matmuls into fp32 psum);
#           col 128 is S_j = sum_m exp, i.e. the kk softmax denominator
#   aggN = agg / S_j                           (row scale, bf16)
#   qT   = transpose(q-chunks)                 (PE transpose, fp32)
#   s1T  = (q @ a)^T  [j, n] via lhsT=a, rhs=qT              (fp32)
#   e1   = exp(s1T) -> bf16
#   T_n  = ones^T @ e1  (column sums = qq softmax denominator)
#   e1n  = e1 * (1/T) broadcast                (bf16)
#   outT = aggN^T-style matmul: lhsT=aggN, rhs=e1n -> [v, n] fp32 -> DRAM
# Host transposes the [H, d, N] per-core outputs back to [H, N, d].

import numpy as np

B, H, N, D = 8, 16, 2048, 128
NCORES = 8

CONFIG = {
    "v_perm": True,     # permuted v load (8KB DMA lines) + strided k lhsT
    "trace": False,
}

_PROGRAM_CACHE = {}


def _patch_tile_drain():
    """This container's walrus rejects >1 sync-wait on a Drain instruction
    (CoreV3GenImpl setupSyncWait). Split the TileContext tail-drain's waits
    across consecutive single-wait drains on the same engine; semantics are
    identical (program order ANDs the waits)."""
    import concourse.tile as tile_mod
    from concourse import mybir
    from concourse.tile import ScopedClock

    if getattr(tile_mod.TileContext, "_agentattn_drain_patched", False):
        return

    def _drain_and_barrier(self, tick_clock, wait_clock):
        nc = self.nc
        drain_inst = nc.sync.drain()
        wait_clock.add_sem_waits(
            drain_inst.ins, ScopedClock({None: tick_clock.global_clock})
        )
        si = drain_inst.ins.sync_info
        if si is not None and si.on_wait and len(si.on_wait) > 1:
            waits = list(si.on_wait)
            ups = list(si.on_update or [])
            drain_inst.ins.sync_info = mybir.SyncInfo(
                on_wait=waits[:1], on_update=ups
            )
            for w in waits[1:]:
                d2 = nc.sync.drain()
                d2.ins.sync_info = mybir.SyncInfo(on_wait=[w], on_update=[])
        nc.all_engine_barrier()
        assert self.sems is not None
        popped = nc._tile_sem_poison_stack.pop()
        assert popped is self._sem_poison
        nc.clear_and_free_semaphores(list(self.sems.allocated().values()))
        nc.all_engine_barrier()

    tile_mod.TileContext._drain_and_barrier = _drain_and_barrier
    tile_mod.TileContext._agentattn_drain_patched = True


def _split_sync_waits(nc, max_waits=1):
    """This container's walrus rejects instructions carrying more than one
    sync-wait command. Hoist excess waits onto same-engine NOPs inserted
    immediately before the instruction (program order on the engine ANDs
    the waits, so semantics are unchanged)."""
    from concourse import mybir

    n_split = 0
    for fn in nc.m.functions:
        for blk in fn.blocks:
            insts = blk.instructions
            if not any(
                (si := inst.sync_info) is not None
                and si.on_wait
                and len(si.on_wait) > max_waits
                for inst in insts
            ):
                continue
            new = []
            for inst in insts:
                si = inst.sync_info
                if si is not None and si.on_wait and len(si.on_wait) > max_waits:
                    waits = list(si.on_wait)
                    for idx, w in enumerate(waits[:-max_waits]):
                        nop = mybir.InstNoOp(
                            name=f"{inst.name}_hw{idx}", ins=[], outs=[]
                        )
                        nop.engine = inst.engine
                        nop.sync_info = mybir.SyncInfo(on_wait=[w], on_update=[])
                        nc.register_instruction(nop)
                        new.append(nop)
                        n_split += 1
                    inst.sync_info = mybir.SyncInfo(
                        on_wait=waits[-max_waits:],
                        on_update=list(si.on_update or []),
                    )
                new.append(inst)
            blk.instructions = new
    return n_split


def install_ntff_hook():
    """Make trace=True work in this container: provide the antenv.axon_hooks
    shim that run_bass_kernel_spmd expects, backed by the injected
    libaxon_pjrt.so, and stub out the artifact upload."""
    import sys, types
    if "antenv.axon_hooks" not in sys.modules:
        from trn_agent_boot.trn_boot import _ntff_profile_via_ctypes
        hook = _ntff_profile_via_ctypes("/opt/axon/libaxon_pjrt.so")
        mod = types.ModuleType("antenv.axon_hooks")
        mod.get_axon_ntff_profile_hook = lambda: hook
        mod.set_axon_ntff_profile_hook = lambda h: None
        sys.modules["antenv.axon_hooks"] = mod
    import concourse.bass_utils as bu
    bu.upload_artifacts = lambda tmpdir: tmpdir


def build_program(cfg=None):
    """Build the single-core Bass program (16 heads of agent attention).

    The host pre-bakes every layout the device wants (see kernel()):
      q  -> qT  [H, D, N]        fp16  (q transposed: s1's moving operand)
      a  ->  a  [H, D, D]        fp16  (s1's stationary operand)
          -> aT [H, D, D]        fp16  (s2's moving operand)
      k  ->  k  [H, D, N]        fp16  (s2's stationary chunks)
      v  -> vp  [H, 128, 16, 129] bf16 (row-permuted n = p*16+c, a ones
                                        column after every 128 values: the
                                        agg matmul's column 128 then yields
                                        the kk softmax denominator S_j)
    Logit matmuls (a@k, q@a) run in fp16 (11-bit mantissa, 1 PE cycle/row
    at any moving width). Value matmuls (e2@v, e1@aggN) run in bf16
    because the un-max-subtracted exp values exceed fp16 range. The fp16
    output is upcast to fp32 on the host.
    """
    import concourse.bass as bass
    import concourse.tile as tile
    from concourse import mybir
    from contextlib import ExitStack

    if cfg is None:
        cfg = CONFIG
    _patch_tile_drain()

    f32 = mybir.dt.float32
    f16 = mybir.dt.float16
    bf16 = mybir.dt.bfloat16
    EXP = mybir.ActivationFunctionType.Exp
    MUL = mybir.AluOpType.mult

    NCH = N // D  # 16 chunks of 128 along the sequence dim

    nc = bass.Bass("TRN2", target_bir_lowering=False, debug=False)
    q_d = nc.dram_tensor("q", [H, D, N], f16, kind="ExternalInput").ap()
    a_d = nc.dram_tensor("a", [H, D, D], f16, kind="ExternalInput").ap()
    aT_d = nc.dram_tensor("at", [H, D, D], f16, kind="ExternalInput").ap()
    k_d = nc.dram_tensor("k", [H, D, N], f16, kind="ExternalInput").ap()
    v_d = nc.dram_tensor("v", [H, 128, NCH, D + 1], bf16, kind="ExternalInput").ap()
    o_d = nc.dram_tensor("o", [H, N, D], f16, kind="ExternalOutput").ap()

    with tile.TileContext(nc) as tc, ExitStack() as ctx:
        p_a = ctx.enter_context(tc.tile_pool(name="p_a", bufs=2))
        p_aT = ctx.enter_context(tc.tile_pool(name="p_aT", bufs=2))
        p_k = ctx.enter_context(tc.tile_pool(name="p_k", bufs=3))
        p_qT = ctx.enter_context(tc.tile_pool(name="p_qT", bufs=3))
        p_e2 = ctx.enter_context(tc.tile_pool(name="p_e2", bufs=3))
        p_vbf = ctx.enter_context(tc.tile_pool(name="p_vbf", bufs=3))
        p_e1 = ctx.enter_context(tc.tile_pool(name="p_e1", bufs=3))
        p_o = ctx.enter_context(tc.tile_pool(name="p_o", bufs=3))
        p_sm = ctx.enter_context(tc.tile_pool(name="p_sm", bufs=3))

        ps_work = ctx.enter_context(tc.tile_pool(name="ps_work", bufs=3, space="PSUM"))
        ps_aggp = ctx.enter_context(tc.tile_pool(name="ps_agg", bufs=2, space="PSUM"))
        ps_out = ctx.enter_context(tc.tile_pool(name="ps_out", bufs=3, space="PSUM"))

        def load_head(h):
            a_sb = p_a.tile([D, D], f16, tag="a")
            nc.sync.dma_start(a_sb, a_d[h])
            aT_sb = p_aT.tile([D, D], f16, tag="aT")
            nc.sync.dma_start(aT_sb, aT_d[h])
            k_sb = p_k.tile([D, N], f16, tag="k")
            nc.sync.dma_start(k_sb, k_d[h])
            v_bf = p_vbf.tile([128, NCH, D + 1], bf16, tag="vbf")
            nc.sync.dma_start(v_bf, v_d[h])
            qT_sb = p_qT.tile([128, N], f16, tag="qT")
            nc.sync.dma_start(qT_sb, q_d[h])
            return a_sb, aT_sb, k_sb, v_bf, qT_sb

        tiles0 = load_head(0)

        for h in range(H):
            a_sb, aT_sb, k_sb, v_bf, qT_sb = tiles0 if h == 0 else load_head(h)

            # ---------------- k side: kk = softmax(a@k), agg = kk @ v ----
            # s2T[m, j] = sum_i k[i, m] a[j, i]; psum quarters of 4 chunks.
            # k chunks are strided slices (m = p*16 + mi) to match the
            # row-permuted v layout.
            e2_sb = p_e2.tile([128, N], bf16, tag="e2")
            for qt in range(4):
                ps = ps_work.tile([128, 512], f32, tag="work")
                for t in range(4):
                    mi = qt * 4 + t
                    nc.tensor.matmul(
                        ps[:, t * D : (t + 1) * D],
                        lhsT=k_sb[:, mi::NCH], rhs=aT_sb,
                        start=True, stop=True,
                    )
                nc.scalar.activation(e2_sb[:, qt * 512 : (qt + 1) * 512], ps, EXP)

            # agg[j, 0:128] = sum_m e2[m, j] v[m, :];  agg[j, 128] = S_j
            # (via the ones column baked into vp on the host)
            agg = ps_aggp.tile([128, D + 1], f32, tag="agg")
            for mi in range(NCH):
                nc.tensor.matmul(
                    agg,
                    lhsT=e2_sb[:, mi * D : (mi + 1) * D],
                    rhs=v_bf[:, mi, :],
                    start=(mi == 0), stop=(mi == NCH - 1),
                )
            recipS = p_sm.tile([128, 1], f32, tag="recipS")
            nc.vector.reciprocal(recipS, agg[:, D : D + 1])
            # aggN has a trailing ones column: the output matmul then yields
            # T_n (the qq softmax denominator) in its own column 128.
            aggN = p_sm.tile([128, D + 1], bf16, tag="aggN")
            nc.gpsimd.memset(aggN[:, D : D + 1], 1.0)
            nc.vector.tensor_tensor(
                aggN[:, 0:D], agg[:, 0:D], recipS.to_broadcast((128, D)), MUL
            )

            # ---------------- q side: qq = softmax(q@a) -------------------
            # s1T[j, n] = sum_i a[i, j] qT[i, n]; qT comes pre-transposed
            # from DRAM with n = p*16 + c; the same permutation flows
            # through e1/out and is undone by the output DMA's pattern.
            e1_sb = p_e1.tile([128, N], bf16, tag="e1")
            for c in range(4):
                ps = ps_work.tile([128, 512], f32, tag="work")
                nc.tensor.matmul(
                    ps, lhsT=a_sb, rhs=qT_sb[:, c * 512 : (c + 1) * 512],
                    start=True, stop=True,
                )
                nc.scalar.activation(e1_sb[:, c * 512 : (c + 1) * 512], ps, EXP)

            # out[n, v] = (sum_j e1[j, n] aggN[j, v]) / T_n; the ones column
            # of aggN makes column 128 of each product chunk equal T_n.
            # Three 129-wide chunks share one psum bank; one strided
            # reciprocal covers the bank's three T columns.
            o_sb = p_o.tile([128, NCH, D], f16, tag="o_sb")
            GRP = [(0, 3), (3, 3), (6, 3), (9, 3), (12, 3), (15, 1)]
            for g0, gn in GRP:
                pso = ps_out.tile([128, 512], f32, tag="out")
                for i in range(gn):
                    ni = g0 + i
                    nc.tensor.matmul(
                        pso[:, i * (D + 1) : (i + 1) * (D + 1)],
                        lhsT=e1_sb[:, ni * D : (ni + 1) * D], rhs=aggN,
                        start=True, stop=True,
                    )
                rcT = p_sm.tile([128, 3], f32, tag="rcT")
                nc.vector.reciprocal(rcT[:, :gn], pso[:, D :: D + 1][:, :gn])
                # one 3D multiply per psum bank: out[:, g, :] = pso[:, g, 0:D]
                # * rcT[:, g] (inner dim broadcast via stride-0)
                nc.vector.tensor_tensor(
                    o_sb[:, g0 : g0 + gn, :],
                    pso[:, 0 : gn * (D + 1)].rearrange(
                        "p (g c) -> p g c", c=D + 1
                    )[:, :, 0:D],
                    rcT[:, 0:gn].to_broadcast((128, gn, D)),
                    MUL,
                )
                nc.sync.dma_start(
                    o_d[h].rearrange("(p c) f -> p c f", p=128)[:, g0 : g0 + gn, :],
                    o_sb[:, g0 : g0 + gn, :],
                )

    _split_sync_waits(nc)
    return nc


def _get_program(cfg_key):
    if cfg_key not in _PROGRAM_CACHE:
        _PROGRAM_CACHE[cfg_key] = build_program()
    return _PROGRAM_CACHE[cfg_key]


def kernel(q, a, k, v):
    from concourse.bass_utils import run_bass_kernel_spmd

    import ml_dtypes

    # Device I/O is 16-bit and pre-laid-out: the kernel contracts logits in
    # fp16 and values in bf16 anyway, so rounding + transposing on the host
    # halves HBM traffic and removes all on-chip transposes at no extra
    # precision cost. The fp16 output is upcast back to fp32 below.
    assert np.asarray(q).shape == (B, H, N, D)
    # qT's free dim uses the same row permutation as the output store
    # (position c*128 + p holds query row n = p*16 + c)
    qt = np.ascontiguousarray(
        np.asarray(q, dtype=np.float16)
        .reshape(B, H, 128, N // 128, D)
        .transpose(0, 1, 4, 3, 2)
        .reshape(B, H, D, N)
    )
    a = np.asarray(a, dtype=np.float16)
    at = np.ascontiguousarray(a.transpose(0, 1, 3, 2))
    a = np.ascontiguousarray(a)
    k = np.ascontiguousarray(np.asarray(k, dtype=np.float16))
    # v: rows permuted to n = p*16 + c, a 1.0 column after every 128 values
    # (feeds the kk softmax denominator out of the agg matmul)
    vp = np.ones((B, H, 128, N // 128, D + 1), dtype=ml_dtypes.bfloat16)
    vp[..., 0:D] = np.asarray(v, dtype=ml_dtypes.bfloat16).reshape(
        B, H, 128, N // 128, D
    )
    vp = np.ascontiguousarray(vp)

    nc = _get_program(("main",))
    core_ids = list(range(NCORES))
    in_maps = [
        {"q": qt[c], "a": a[c], "at": at[c], "k": k[c], "v": vp[c]}
        for c in core_ids
    ]
    res = run_bass_kernel_spmd(nc, in_maps, core_ids, trace=CONFIG["trace"])
    out = np.stack(
        [np.asarray(res.results[c]["o"], dtype=np.float32) for c in core_ids]
    )
    kernel.last_result = res
    return out



# revision 23
# speedup vs baseline: 1.1271x; 1.1271x over previous
# kernel.py — AgentAttention on 8 Trainium2 NeuronCores (self-contained).
#
# Problem (per batch b, head h):
#   qq  = softmax(q @ a, axis=-1)            # [N, d] over agents d
#   kk  = softmax(a @ k, axis=-1)            # [d, N] over keys N
#   out = qq @ (kk @ v)                      # [N, d]
# Shapes: q [8,16,2048,128], a [8,16,128,128], k [8,16,128,2048],
#         v [8,16,2048,128]; d == n_agents == 128.
#
# Sharding: batch dimension (8) across the 8 cores; each core computes its
# 16 heads independently (pure data parallel, no collectives).
#
# Per-head device algorithm (all matmuls contract over the partition dim):
#   aT   = transpose(a)                        (PE transpose, fp32)
#   s2T  = (a @ k)^T  [m, j] via lhsT=k-chunk, rhs=aT        (fp32)
#   e2   = exp(s2T)  -> bf16   (no max subtraction: |logit| <~ 70 < 88.7,
#                               exp fits fp32/bf16 range exactly)
#   agg|S = sum_m e2[m,:]^T @ [v_m | 1]  (bf16 matmuls into fp32 psum);
#           col 128 is S_j = sum_m exp, i.e. the kk softmax denominator
#   aggN = agg / S_j                           (row scale, bf16)
#   qT   = transpose(q-chunks)                 (PE transpose, fp32)
#   s1T  = (q @ a)^T  [j, n] via lhsT=a, rhs=qT              (fp32)
#   e1   = exp(s1T) -> bf16
#   T_n  = ones^T @ e1  (column sums = qq softmax denominator)
#   e1n  = e1 * (1/T) broadcast                (bf16)
#   outT = aggN^T-style matmul: lhsT=aggN, rhs=e1n -> [v, n] fp32 -> DRAM
# Host transposes the [H, d, N] per-core outputs back to [H, N, d].

import numpy as np

B, H, N, D = 8, 16, 2048, 128
NCORES = 8

CONFIG = {
    "v_perm": True,     # permuted v load (8KB DMA lines) + strided k lhsT
    "trace": False,
}

_PROGRAM_CACHE = {}


def _patch_tile_drain():
    """This container's walrus rejects >1 sync-wait on a Drain instruction
    (CoreV3GenImpl setupSyncWait). Split the TileContext tail-drain's waits
    across consecutive single-wait drains on the same engine; semantics are
    identical (program order ANDs the waits)."""
    import concourse.tile as tile_mod
    from concourse import mybir
    from concourse.tile import ScopedClock

    if getattr(tile_mod.TileContext, "_agentattn_drain_patched", False):
        return

    def _drain_and_barrier(self, tick_clock, wait_clock):
        nc = self.nc
        drain_inst = nc.sync.drain()
        wait_clock.add_sem_waits(
            drain_inst.ins, ScopedClock({None: tick_clock.global_clock})
        )
        si = drain_inst.ins.sync_info
        if si is not None and si.on_wait and len(si.on_wait) > 1:
            waits = list(si.on_wait)
            ups = list(si.on_update or [])
            drain_inst.ins.sync_info = mybir.SyncInfo(
                on_wait=waits[:1], on_update=ups
            )
            for w in waits[1:]:
                d2 = nc.sync.drain()
                d2.ins.sync_info = mybir.SyncInfo(on_wait=[w], on_update=[])
        nc.all_engine_barrier()
        assert self.sems is not None
        popped = nc._tile_sem_poison_stack.pop()
        assert popped is self._sem_poison
        nc.clear_and_free_semaphores(list(self.sems.allocated().values()))
        nc.all_engine_barrier()

    tile_mod.TileContext._drain_and_barrier = _drain_and_barrier
    tile_mod.TileContext._agentattn_drain_patched = True


def _split_sync_waits(nc, max_waits=1):
    """This container's walrus rejects instructions carrying more than one
    sync-wait command. Hoist excess waits onto same-engine NOPs inserted
    immediately before the instruction (program order on the engine ANDs
    the waits, so semantics are unchanged)."""
    from concourse import mybir

    n_split = 0
    for fn in nc.m.functions:
        for blk in fn.blocks:
            insts = blk.instructions
            if not any(
                (si := inst.sync_info) is not None
                and si.on_wait
                and len(si.on_wait) > max_waits
                for inst in insts
            ):
                continue
            new = []
            for inst in insts:
                si = inst.sync_info
                if si is not None and si.on_wait and len(si.on_wait) > max_waits:
                    waits = list(si.on_wait)
                    for idx, w in enumerate(waits[:-max_waits]):
                        nop = mybir.InstNoOp(
                            name=f"{inst.name}_hw{idx}", ins=[], outs=[]
                        )
                        nop.engine = inst.engine
                        nop.sync_info = mybir.SyncInfo(on_wait=[w], on_update=[])
                        nc.register_instruction(nop)
                        new.append(nop)
                        n_split += 1
                    inst.sync_info = mybir.SyncInfo(
                        on_wait=waits[-max_waits:],
                        on_update=list(si.on_update or []),
                    )
                new.append(inst)
            blk.instructions = new
    return n_split


def install_ntff_hook():
    """Make trace=True work in this container: provide the antenv.axon_hooks
    shim that run_bass_kernel_spmd expects, backed by the injected
    libaxon_pjrt.so, and stub out the artifact upload."""
    import sys, types
    if "antenv.axon_hooks" not in sys.modules:
        from trn_agent_boot.trn_boot import _ntff_profile_via_ctypes
        hook = _ntff_profile_via_ctypes("/opt/axon/libaxon_pjrt.so")
        mod = types.ModuleType("antenv.axon_hooks")
        mod.get_axon_ntff_profile_hook = lambda: hook
        mod.set_axon_ntff_profile_hook = lambda h: None
        sys.modules["antenv.axon_hooks"] = mod
    import concourse.bass_utils as bu
    bu.upload_artifacts = lambda tmpdir: tmpdir


def build_program(cfg=None):
    """Build the single-core Bass program (16 heads of agent attention).

    The host pre-bakes every layout the device wants (see kernel()):
      q  -> qT  [H, D, N]        fp16  (q transposed: s1's moving operand)
      a  ->  a  [H, D, D]        fp16  (s1's stationary operand)
          -> aT [H, D, D]        fp16  (s2's moving operand)
      k  ->  k  [H, D, N]        fp16  (s2's stationary chunks)
      v  -> vp  [H, 128, 16, 129] bf16 (row-permuted n = p*16+c, a ones
                                        column after every 128 values: the
                                        agg matmul's column 128 then yields
                                        the kk softmax denominator S_j)
    Logit matmuls (a@k, q@a) run in fp16 (11-bit mantissa, 1 PE cycle/row
    at any moving width). Value matmuls (e2@v, e1@aggN) run in bf16
    because the un-max-subtracted exp values exceed fp16 range. The fp16
    output is upcast to fp32 on the host.
    """
    import concourse.bass as bass
    import concourse.tile as tile
    from concourse import mybir
    from contextlib import ExitStack

    if cfg is None:
        cfg = CONFIG
    _patch_tile_drain()

    f32 = mybir.dt.float32
    f16 = mybir.dt.float16
    bf16 = mybir.dt.bfloat16
    EXP = mybir.ActivationFunctionType.Exp
    MUL = mybir.AluOpType.mult

    NCH = N // D  # 16 chunks of 128 along the sequence dim

    nc = bass.Bass("TRN2", target_bir_lowering=False, debug=False)
    q_d = nc.dram_tensor("q", [H, D, N], f16, kind="ExternalInput").ap()
    a_d = nc.dram_tensor("a", [H, D, D], f16, kind="ExternalInput").ap()
    aT_d = nc.dram_tensor("at", [H, D, D], f16, kind="ExternalInput").ap()
    k_d = nc.dram_tensor("k", [H, D, N], f16, kind="ExternalInput").ap()
    v_d = nc.dram_tensor("v", [H, 128, NCH, D + 1], bf16, kind="ExternalInput").ap()
    o_d = nc.dram_tensor("o", [H, N, D], f16, kind="ExternalOutput").ap()

    with tile.TileContext(nc) as tc, ExitStack() as ctx:
        p_a = ctx.enter_context(tc.tile_pool(name="p_a", bufs=4))
        p_aT = ctx.enter_context(tc.tile_pool(name="p_aT", bufs=4))
        p_k = ctx.enter_context(tc.tile_pool(name="p_k", bufs=3))
        p_qT = ctx.enter_context(tc.tile_pool(name="p_qT", bufs=3))
        p_e2 = ctx.enter_context(tc.tile_pool(name="p_e2", bufs=3))
        p_vbf = ctx.enter_context(tc.tile_pool(name="p_vbf", bufs=3))
        p_e1 = ctx.enter_context(tc.tile_pool(name="p_e1", bufs=3))
        p_o = ctx.enter_context(tc.tile_pool(name="p_o", bufs=3))
        p_sm = ctx.enter_context(tc.tile_pool(name="p_sm", bufs=3))

        ps_work = ctx.enter_context(tc.tile_pool(name="ps_work", bufs=3, space="PSUM"))
        ps_aggp = ctx.enter_context(tc.tile_pool(name="ps_agg", bufs=2, space="PSUM"))
        ps_out = ctx.enter_context(tc.tile_pool(name="ps_out", bufs=3, space="PSUM"))

        def load_head(h):
            # two independent HWDGE FIFOs (sync + scalar) so a stalled tile
            # wait on one ring can't head-of-line-block the other loads;
            # s2's inputs (k, aT) go out first
            k_sb = p_k.tile([D, N], f16, tag="k")
            nc.sync.dma_start(k_sb, k_d[h])
            aT_sb = p_aT.tile([D, D], f16, tag="aT")
            nc.scalar.dma_start(aT_sb, aT_d[h])
            v_bf = p_vbf.tile([128, NCH, D + 1], bf16, tag="vbf")
            nc.sync.dma_start(v_bf, v_d[h])
            qT_sb = p_qT.tile([128, N], f16, tag="qT")
            nc.scalar.dma_start(qT_sb, q_d[h])
            a_sb = p_a.tile([D, D], f16, tag="a")
            nc.scalar.dma_start(a_sb, a_d[h])
            return a_sb, aT_sb, k_sb, v_bf, qT_sb

        tiles0 = load_head(0)

        for h in range(H):
            a_sb, aT_sb, k_sb, v_bf, qT_sb = tiles0 if h == 0 else load_head(h)

            # ---------------- k side: kk = softmax(a@k), agg = kk @ v ----
            # s2T[m, j] = sum_i k[i, m] a[j, i]; psum quarters of 4 chunks.
            # k chunks are strided slices (m = p*16 + mi) to match the
            # row-permuted v layout.
            e2_sb = p_e2.tile([128, N], bf16, tag="e2")
            for qt in range(4):
                ps = ps_work.tile([128, 512], f32, tag="work")
                for t in range(4):
                    mi = qt * 4 + t
                    nc.tensor.matmul(
                        ps[:, t * D : (t + 1) * D],
                        lhsT=k_sb[:, mi::NCH], rhs=aT_sb,
                        start=True, stop=True,
                    )
                nc.scalar.activation(e2_sb[:, qt * 512 : (qt + 1) * 512], ps, EXP)

            # agg[j, 0:128] = sum_m e2[m, j] v[m, :];  agg[j, 128] = S_j
            # (via the ones column baked into vp on the host)
            agg = ps_aggp.tile([128, D + 1], f32, tag="agg")
            for mi in range(NCH):
                nc.tensor.matmul(
                    agg,
                    lhsT=e2_sb[:, mi * D : (mi + 1) * D],
                    rhs=v_bf[:, mi, :],
                    start=(mi == 0), stop=(mi == NCH - 1),
                )
            recipS = p_sm.tile([128, 1], f32, tag="recipS")
            nc.vector.reciprocal(recipS, agg[:, D : D + 1])
            # aggN has a trailing ones column: the output matmul then yields
            # T_n (the qq softmax denominator) in its own column 128.
            aggN = p_sm.tile([128, D + 1], bf16, tag="aggN")
            nc.gpsimd.memset(aggN[:, D : D + 1], 1.0)
            nc.vector.tensor_tensor(
                aggN[:, 0:D], agg[:, 0:D], recipS.to_broadcast((128, D)), MUL
            )

            # ---------------- q side: qq = softmax(q@a) -------------------
            # s1T[j, n] = sum_i a[i, j] qT[i, n]; qT comes pre-transposed
            # from DRAM with n = p*16 + c; the same permutation flows
            # through e1/out and is undone by the output DMA's pattern.
            e1_sb = p_e1.tile([128, N], bf16, tag="e1")
            for c in range(4):
                ps = ps_work.tile([128, 512], f32, tag="work")
                nc.tensor.matmul(
                    ps, lhsT=a_sb, rhs=qT_sb[:, c * 512 : (c + 1) * 512],
                    start=True, stop=True,
                )
                nc.scalar.activation(e1_sb[:, c * 512 : (c + 1) * 512], ps, EXP)

            # out[n, v] = (sum_j e1[j, n] aggN[j, v]) / T_n; the ones column
            # of aggN makes column 128 of each product chunk equal T_n.
            # Three 129-wide chunks share one psum bank; one strided
            # reciprocal covers the bank's three T columns.
            o_sb = p_o.tile([128, NCH, D], f16, tag="o_sb")
            GRP = [(0, 3), (3, 3), (6, 3), (9, 3), (12, 3), (15, 1)]
            for g0, gn in GRP:
                pso = ps_out.tile([128, 512], f32, tag="out")
                for i in range(gn):
                    ni = g0 + i
                    nc.tensor.matmul(
                        pso[:, i * (D + 1) : (i + 1) * (D + 1)],
                        lhsT=e1_sb[:, ni * D : (ni + 1) * D], rhs=aggN,
                        start=True, stop=True,
                    )
                rcT = p_sm.tile([128, 3], f32, tag="rcT")
                nc.vector.reciprocal(rcT[:, :gn], pso[:, D :: D + 1][:, :gn])
                # one 3D multiply per psum bank: out[:, g, :] = pso[:, g, 0:D]
                # * rcT[:, g] (inner dim broadcast via stride-0)
                nc.vector.tensor_tensor(
                    o_sb[:, g0 : g0 + gn, :],
                    pso[:, 0 : gn * (D + 1)].rearrange(
                        "p (g c) -> p g c", c=D + 1
                    )[:, :, 0:D],
                    rcT[:, 0:gn].to_broadcast((128, gn, D)),
                    MUL,
                )
                nc.sync.dma_start(
                    o_d[h].rearrange("(p c) f -> p c f", p=128)[:, g0 : g0 + gn, :],
                    o_sb[:, g0 : g0 + gn, :],
                )

    _split_sync_waits(nc)
    return nc


def _get_program(cfg_key):
    if cfg_key not in _PROGRAM_CACHE:
        _PROGRAM_CACHE[cfg_key] = build_program()
    return _PROGRAM_CACHE[cfg_key]


def kernel(q, a, k, v):
    from concourse.bass_utils import run_bass_kernel_spmd

    import ml_dtypes

    # Device I/O is 16-bit and pre-laid-out: the kernel contracts logits in
    # fp16 and values in bf16 anyway, so rounding + transposing on the host
    # halves HBM traffic and removes all on-chip transposes at no extra
    # precision cost. The fp16 output is upcast back to fp32 below.
    assert np.asarray(q).shape == (B, H, N, D)
    # qT's free dim uses the same row permutation as the output store
    # (position c*128 + p holds query row n = p*16 + c)
    qt = np.ascontiguousarray(
        np.asarray(q, dtype=np.float16)
        .reshape(B, H, 128, N // 128, D)
        .transpose(0, 1, 4, 3, 2)
        .reshape(B, H, D, N)
    )
    a = np.asarray(a, dtype=np.float16)
    at = np.ascontiguousarray(a.transpose(0, 1, 3, 2))
    a = np.ascontiguousarray(a)
    k = np.ascontiguousarray(np.asarray(k, dtype=np.float16))
    # v: rows permuted to n = p*16 + c, a 1.0 column after every 128 values
    # (feeds the kk softmax denominator out of the agg matmul)
    vp = np.ones((B, H, 128, N // 128, D + 1), dtype=ml_dtypes.bfloat16)
    vp[..., 0:D] = np.asarray(v, dtype=ml_dtypes.bfloat16).reshape(
        B, H, 128, N // 128, D
    )
    vp = np.ascontiguousarray(vp)

    nc = _get_program(("main",))
    core_ids = list(range(NCORES))
    in_maps = [
        {"q": qt[c], "a": a[c], "at": at[c], "k": k[c], "v": vp[c]}
        for c in core_ids
    ]
    res = run_bass_kernel_spmd(nc, in_maps, core_ids, trace=CONFIG["trace"])
    out = np.stack(
        [np.asarray(res.results[c]["o"], dtype=np.float32) for c in core_ids]
    )
    kernel.last_result = res
    return out

